# revision 1
# baseline (speedup 1.0000x reference)
"""DeepHisCoM forward pass on 8 Trainium2 NeuronCores.

Strategy: pathway (expert) parallelism — 8 of the 64 pathways per core.
Pathway blocks are independent until the final concat, and BatchNorm's
batch statistics are per-pathway, so they stay core-local. The only
cross-core data needed is (a) the global L2 norm's sum of squares and
(b) the final linear layer's pathway partial dot products — both linear
in pn, so a single [2049]-float AllReduce carries everything.

Host side pre-shards and pre-transposes x into feature-major bf16 per
core (the PE contracts along partitions, so activations must be
feature-major; doing the transpose on host costs no device time).
bf16 GEMMs run 4x faster on the PE than fp32 and are numerically safe
here: BatchNorm + the global L2 norm renormalize each pathway column,
so bf16 rounding (~0.4%) enters the final logits scaled by 1/||pn||
(~1/362) times 0.05-scale fc weights — ~1e-5 absolute.
"""

import os
import sys

sys.path.insert(0, "/opt/trn_rl_repo")

from contextlib import ExitStack

import ml_dtypes
import numpy as np

import concourse.bacc as bacc
import concourse.bass as bass
import concourse.tile as tile
from concourse import mybir
from concourse.bass_utils import run_bass_kernel_spmd

P_TOT = 64   # pathways
NV = 512     # features per pathway
WID = 256    # hidden width
COV = 16     # covariates
B = 2048     # batch
EPS = 1e-5
SLOPE = 0.2
NCORES = 8
PPC = P_TOT // NCORES  # pathways per core
KT1 = NV // 128        # k-tiles for GEMM1
KT2 = WID // 128       # k-tiles for GEMM2 / GEMV
MT = WID // 128        # m-tiles (output feature tiles)
NCH = B // 512         # batch chunks of 512

BF16 = mybir.dt.bfloat16
F32 = mybir.dt.float32
AF = mybir.ActivationFunctionType
ALU = mybir.AluOpType

# Native Lrelu runs on hardware; the CPU interpreter doesn't implement it,
# so sim checks set KERNEL_LRELU=0 to use the max(x, 0.2x) fallback.
USE_NATIVE_LRELU = os.environ.get("KERNEL_LRELU", "1") == "1"


def _lrelu_evict(nc, sc_pool, ps, dst):
    """dst = leaky_relu(ps); ps is a PSUM tile viewed [128, free]."""
    ps2 = ps.rearrange("p a b -> p (a b)")
    if USE_NATIVE_LRELU:
        nc.scalar.activation(dst, ps2, AF.Lrelu, alpha=SLOPE)
    else:
        free = ps2.shape[1]
        sc = sc_pool.tile([128, free], F32, tag="sc", name="sc")
        nc.scalar.activation(sc[:], ps2, AF.Copy, scale=SLOPE)
        nc.vector.tensor_tensor(dst, ps2, sc[:], ALU.max)


def _emit(ctx, tc, xt, w1, w2, w3p, xcovt, fcwp, fcwc, fcb, gam, bet, out):
    nc = tc.nc

    xt_pool = ctx.enter_context(tc.tile_pool(name="xt_pool", bufs=2))
    w_pool = ctx.enter_context(tc.tile_pool(name="w_pool", bufs=2))
    h1_pool = ctx.enter_context(tc.tile_pool(name="h1_pool", bufs=2))
    h2_pool = ctx.enter_context(
        tc.tile_pool(name="h2_pool", bufs=5 if USE_NATIVE_LRELU else 4))
    sc_pool = ctx.enter_context(tc.tile_pool(name="sc_pool", bufs=2))
    vt_pool = ctx.enter_context(tc.tile_pool(name="vt_pool", bufs=2))
    gs_pool = ctx.enter_context(tc.tile_pool(name="gs_pool", bufs=2))
    one = ctx.enter_context(tc.tile_pool(name="one", bufs=1))
    psg = ctx.enter_context(tc.tile_pool(name="psg", bufs=2, space="PSUM"))
    dram = ctx.enter_context(tc.tile_pool(name="dram", bufs=1, space="DRAM"))

    # ---- persistents ----
    w3_sb = one.tile([128, 2, KT2, 128], BF16)
    nc.sync.dma_start(out=w3_sb[:],
                      in_=w3p.rearrange("g (kt kp) m -> kp g kt m", kp=128))
    xcov_sb = one.tile([COV, B], BF16)
    nc.sync.dma_start(out=xcov_sb[:], in_=xcovt[:])
    # Engine APs must start at partition 0/32/64/96, so the 8 pathways are
    # laid out as [4 partitions, 2 group columns] (pathway p = g*4 + j).
    fcwp_sb = one.tile([4, 2], BF16)
    nc.sync.dma_start(out=fcwp_sb[:],
                      in_=fcwp.rearrange("(g j) one -> j (g one)", j=4))
    fcwc_sb = one.tile([COV, 1], BF16)
    nc.sync.dma_start(out=fcwc_sb[:], in_=fcwc[:])
    fcb_sb = one.tile([1, 1], F32)
    nc.sync.dma_start(out=fcb_sb[:], in_=fcb[:])
    gam_sb = one.tile([4, 2], F32)
    nc.sync.dma_start(out=gam_sb[:],
                      in_=gam.rearrange("(g j) one -> j (g one)", j=4))
    bet_sb = one.tile([4, 2], F32)
    nc.sync.dma_start(out=bet_sb[:],
                      in_=bet.rearrange("(g j) one -> j (g one)", j=4))
    ones_sb = one.tile([4, 1], BF16)
    nc.vector.memset(ones_sb[:], 1.0)
    eps_sb = one.tile([4, 1], F32)
    nc.vector.memset(eps_sb[:], EPS)

    p_all = one.tile([4, 2, B], F32)
    pn_bf = one.tile([4, 2, B], BF16)
    stats = one.tile([4, 2, NCH, 6], F32)
    mv = one.tile([4, 2, 2], F32)
    rstd = one.tile([4, 2], F32)
    a_sc = one.tile([4, 2], F32)
    b_sc = one.tile([4, 2], F32)
    ssq = one.tile([4, 2], F32)
    ssq_bf = one.tile([4, 1], BF16)
    s_row = one.tile([1, B], F32)
    cov_row = one.tile([1, B], F32)

    # ---- covariate term first: fills the DMA-bound kernel start ----
    for ncol in range(NCH):
        pc = psg.tile([128, 4, 512], F32, tag="g", name="pc")
        nc.tensor.matmul(pc[0:1, 0, :], fcwc_sb[:],
                         xcov_sb[:, ncol * 512:(ncol + 1) * 512],
                         start=True, stop=True)
        nc.scalar.activation(cov_row[:, ncol * 512:(ncol + 1) * 512],
                             pc[0:1, 0, :], AF.Copy)

    def group_tail(g):
        """Per-pathway-group BN chain; group 0's overlaps pathways 4-7."""
        if not USE_NATIVE_LRELU:
            scr = gs_pool.tile([4, B], F32, tag="gscr", name="scr")
            nc.vector.tensor_scalar_mul(scr[:], p_all[:, g, :], SLOPE)
            nc.vector.tensor_tensor(p_all[:, g, :], p_all[:, g, :], scr[:],
                                    ALU.max)
        for s in range(NCH):
            nc.vector.bn_stats(out=stats[:, g, s, :],
                               in_=p_all[:, g, s * 512:(s + 1) * 512])
        nc.vector.bn_aggr(out=mv[:, g, :], in_=stats[:, g])
        nc.scalar.activation(rstd[:, g:g + 1], mv[:, g, 1:2], AF.Sqrt,
                             bias=eps_sb[:])
        nc.vector.reciprocal(rstd[:, g:g + 1], rstd[:, g:g + 1])
        nc.vector.tensor_tensor(a_sc[:, g:g + 1], gam_sb[:, g:g + 1],
                                rstd[:, g:g + 1], ALU.mult)
        nc.vector.tensor_tensor(b_sc[:, g:g + 1], mv[:, g, 0:1],
                                a_sc[:, g:g + 1], ALU.mult)
        nc.vector.tensor_tensor(b_sc[:, g:g + 1], bet_sb[:, g:g + 1],
                                b_sc[:, g:g + 1], ALU.subtract)
        # pn overwrites p_all in place; bf16 copy feeds the final matmuls
        nc.vector.tensor_scalar(p_all[:, g], p_all[:, g], a_sc[:, g:g + 1],
                                b_sc[:, g:g + 1], ALU.mult, ALU.add)
        nc.scalar.activation(pn_bf[:, g], p_all[:, g], AF.Copy)
        sqs = gs_pool.tile([4, B], F32, tag="gsq", name="sqs")
        nc.scalar.activation(sqs[:], p_all[:, g], AF.Square,
                             accum_out=ssq[:, g:g + 1])

    # ---- pathway loop ----
    h2_tiles = []
    for p in range(PPC):
        xt_sb = xt_pool.tile([128, KT1, B], BF16, tag="xt", name="xt_sb")
        nc.sync.dma_start(
            out=xt_sb[:], in_=xt[p].rearrange("(kt kp) b -> kp kt b", kp=128)
        )
        w1_sb = w_pool.tile([128, KT1, WID], BF16, tag="w1", name="w1_sb")
        nc.sync.dma_start(
            out=w1_sb[:], in_=w1[p].rearrange("(kt kp) m -> kp kt m", kp=128)
        )
        w2_sb = w_pool.tile([128, KT2, WID], BF16, tag="w2", name="w2_sb")
        nc.sync.dma_start(
            out=w2_sb[:], in_=w2[p].rearrange("(kt kp) m -> kp kt m", kp=128)
        )

        h1_sb = h1_pool.tile([128, MT, B], BF16, tag="h1", name="h1_sb")
        h2_sb = h2_pool.tile([128, MT, B], BF16, tag="h2", name="h2_sb")

        # GEMM1: h1[o, b] = lrelu(sum_i W1[i, o] * xT[i, b]).
        # One [128,4,512] PSUM tile per m-block: 4 matmuls share each
        # LDWEIGHTS and the eviction is one big ACT op.
        for m in range(MT):
            ps = psg.tile([128, 4, 512], F32, tag="g", name="ps")
            for k in range(KT1):
                for n in range(NCH):
                    nc.tensor.matmul(
                        ps[:, n],
                        w1_sb[:, k, m * 128:(m + 1) * 128],
                        xt_sb[:, k, n * 512:(n + 1) * 512],
                        start=(k == 0),
                        stop=(k == KT1 - 1),
                    )
            _lrelu_evict(nc, sc_pool, ps, h1_sb[:, m, :])

        # GEMM2: h2[o, b] = lrelu(sum_i W2[i, o] * h1[i, b])
        for m in range(MT):
            ps = psg.tile([128, 4, 512], F32, tag="g", name="ps")
            for k in range(KT2):
                for n in range(NCH):
                    nc.tensor.matmul(
                        ps[:, n],
                        w2_sb[:, k, m * 128:(m + 1) * 128],
                        h1_sb[:, k, n * 512:(n + 1) * 512],
                        start=(k == 0),
                        stop=(k == KT2 - 1),
                    )
            _lrelu_evict(nc, sc_pool, ps, h2_sb[:, m, :])
        h2_tiles.append(h2_sb)

        # GEMV3 for a group of 4 pathways, packed into PE column groups
        # (tile_position) so the 4 matmuls run concurrently. W3 is zero-padded
        # to M=32 slabs on host so every PSUM row is written; the eviction
        # copies all 128 rows (free dim drives cost) and a DMA gathers rows
        # {0,32,64,96} to contiguous partitions (engines can't stride
        # partitions, DMA can).
        if p % 4 == 3:
            g = p // 4
            vt = vt_pool.tile([128, B], F32, tag="vt", name="vt")
            for ncol in range(NCH):
                pv = psg.tile([128, 4, 512], F32, tag="g", name="pv")
                pv = pv[:, 0, :]
                for j in range(4):
                    for k in range(KT2):
                        nc.tensor.matmul(
                            pv[32 * j:32 * j + 32, :],
                            w3_sb[:, g, k, 32 * j:32 * j + 32],
                            h2_tiles[g * 4 + j][:, k, ncol * 512:(ncol + 1) * 512],
                            start=(k == 0),
                            stop=(k == KT2 - 1),
                            tile_position=(0, 32 * j),
                        )
                if USE_NATIVE_LRELU:
                    nc.scalar.activation(
                        vt[:, ncol * 512:(ncol + 1) * 512], pv[:], AF.Lrelu,
                        alpha=SLOPE)
                else:
                    nc.scalar.activation(
                        vt[:, ncol * 512:(ncol + 1) * 512], pv[:], AF.Copy)
            nc.sync.dma_start(out=p_all[:, g, :], in_=vt[0:97:32, :])
            group_tail(g)

    # ---- combine: s partials and sum of squares ----
    sp = psg.tile([128, 4, 512], F32, tag="g", name="sp")
    for ncol in range(NCH):
        for g in range(2):
            nc.tensor.matmul(sp[0:1, ncol, :], fcwp_sb[:, g:g + 1],
                             pn_bf[:, g, ncol * 512:(ncol + 1) * 512],
                             start=(g == 0), stop=(g == 1))
    nc.scalar.activation(s_row[:], sp[0:1, :, :], AF.Copy)
    nc.vector.tensor_tensor(ssq[:, 0:1], ssq[:, 0:1], ssq[:, 1:2], ALU.add)
    nc.scalar.activation(ssq_bf[:], ssq[:, 0:1], AF.Copy)
    ss_sb = one.tile([1, 1], F32)
    ppq = psg.tile([128, 4, 512], F32, tag="g", name="ppq")
    nc.tensor.matmul(ppq[0:1, 0, 0:1], ones_sb[:], ssq_bf[:],
                     start=True, stop=True)
    nc.scalar.activation(ss_sb[:], ppq[0:1, 0, 0:1], AF.Copy)

    # one AllReduce for both the 2048 partial dots and the sum of squares
    ar_in = dram.tile([1, B + 1], F32)
    ar_out = dram.tile([1, B + 1], F32)
    nc.sync.dma_start(out=ar_in[0:1, 0:B], in_=s_row[:])
    nc.sync.dma_start(out=ar_in[0:1, B:B + 1], in_=ss_sb[:])
    nc.gpsimd.collective_compute(
        "AllReduce",
        ALU.add,
        replica_groups=[list(range(NCORES))],
        ins=[ar_in.opt()],
        outs=[ar_out.opt()],
    )
    s_tot = one.tile([1, B], F32)
    nc.sync.dma_start(out=s_tot[:], in_=ar_out[0:1, 0:B])
    ss_tot = one.tile([1, 1], F32)
    nc.sync.dma_start(out=ss_tot[:], in_=ar_out[0:1, B:B + 1])

    # 1 / ||pn||
    rn = one.tile([1, 1], F32)
    nc.scalar.activation(rn[:], ss_tot[:], AF.Sqrt)
    nc.vector.reciprocal(rn[:], rn[:])

    # out = sigmoid(s_tot / ||pn|| + cov + fc_b), in place on s_tot/cov_row
    nc.vector.tensor_scalar(s_tot[:], s_tot[:], rn[:], None, ALU.mult)
    nc.vector.tensor_tensor(s_tot[:], s_tot[:], cov_row[:], ALU.add)
    nc.scalar.activation(cov_row[:], s_tot[:], AF.Sigmoid, bias=fcb_sb[:])
    nc.sync.dma_start(out=out.rearrange("b one -> one b"), in_=cov_row[:])


_NC = None


def _get_compiled():
    global _NC
    if _NC is None:
        nc = bacc.Bacc("TRN2", target_bir_lowering=False, debug=False,
                       num_devices=NCORES)
        xt = nc.dram_tensor("xt", [PPC, NV, B], BF16, kind="ExternalInput").ap()
        w1 = nc.dram_tensor("w1", [PPC, NV, WID], BF16, kind="ExternalInput").ap()
        w2 = nc.dram_tensor("w2", [PPC, WID, WID], BF16, kind="ExternalInput").ap()
        w3p = nc.dram_tensor("w3p", [2, WID, 128], BF16, kind="ExternalInput").ap()
        xcovt = nc.dram_tensor("xcovt", [COV, B], BF16, kind="ExternalInput").ap()
        fcwp = nc.dram_tensor("fcwp", [PPC, 1], BF16, kind="ExternalInput").ap()
        fcwc = nc.dram_tensor("fcwc", [COV, 1], BF16, kind="ExternalInput").ap()
        fcb = nc.dram_tensor("fcb", [1, 1], F32, kind="ExternalInput").ap()
        gam = nc.dram_tensor("gam", [PPC, 1], F32, kind="ExternalInput").ap()
        bet = nc.dram_tensor("bet", [PPC, 1], F32, kind="ExternalInput").ap()
        out = nc.dram_tensor("out", [B, 1], F32, kind="ExternalOutput").ap()
        with tile.TileContext(nc) as tc:
            with ExitStack() as ctx:
                _emit(ctx, tc, xt, w1, w2, w3p, xcovt, fcwp, fcwc, fcb, gam,
                      bet, out)
        nc.compile()
        _NC = nc
    return _NC


def _shard(inputs):
    x = np.asarray(inputs["x"], np.float32)
    W1 = np.asarray(inputs["W1"], np.float32)
    W2 = np.asarray(inputs["W2"], np.float32)
    W3 = np.asarray(inputs["W3"], np.float32)
    gamma = np.asarray(inputs["gamma"], np.float32)
    beta = np.asarray(inputs["beta"], np.float32)
    fc_w = np.asarray(inputs["fc_w"], np.float32)
    fc_b = np.asarray(inputs["fc_b"], np.float32)

    xm = x[:, :P_TOT * NV].reshape(B, P_TOT, NV)
    xcovt = np.ascontiguousarray(
        x[:, P_TOT * NV:P_TOT * NV + COV].T).astype(ml_dtypes.bfloat16)
    fcwc = np.ascontiguousarray(
        fc_w[P_TOT:P_TOT + COV].reshape(COV, 1)).astype(ml_dtypes.bfloat16)
    fcb = fc_b.reshape(1, 1).astype(np.float32)

    maps = []
    for c in range(NCORES):
        sl = slice(c * PPC, (c + 1) * PPC)
        xt_c = np.ascontiguousarray(
            xm[:, sl, :].transpose(1, 2, 0)).astype(ml_dtypes.bfloat16)
        w3p_c = np.zeros((2, WID, 128), np.float32)
        for g in range(2):
            for j in range(4):
                w3p_c[g, :, 32 * j] = W3[c * PPC + g * 4 + j]
        w3p_c = w3p_c.astype(ml_dtypes.bfloat16)
        maps.append({
            "xt": xt_c,
            "w1": np.ascontiguousarray(W1[sl]).astype(ml_dtypes.bfloat16),
            "w2": np.ascontiguousarray(W2[sl]).astype(ml_dtypes.bfloat16),
            "w3p": w3p_c,
            "xcovt": xcovt,
            "fcwp": np.ascontiguousarray(
                fc_w[sl].reshape(PPC, 1)).astype(ml_dtypes.bfloat16),
            "fcwc": fcwc,
            "fcb": fcb,
            "gam": np.ascontiguousarray(gamma[sl].reshape(PPC, 1)),
            "bet": np.ascontiguousarray(beta[sl].reshape(PPC, 1)),
        })
    return maps


def kernel(**inputs) -> np.ndarray:
    nc = _get_compiled()
    maps = _shard(inputs)
    res = run_bass_kernel_spmd(nc, maps, list(range(NCORES)))
    return np.asarray(res.results[0]["out"], np.float32)


def kernel_traced(**inputs):
    """Like kernel() but with NTFF profiling; returns (out, BassKernelResults)."""
    nc = _get_compiled()
    maps = _shard(inputs)
    res = run_bass_kernel_spmd(nc, maps, list(range(NCORES)), trace=True)
    return np.asarray(res.results[0]["out"], np.float32), res

